# revision 21
# baseline (speedup 1.0000x reference)
"""Multi-head attention with RoPE (B=4, T=1024, D=1024, H=16) on 8 TRN2
NeuronCores.

Sharding: 64 (batch, head) units -> core c handles batch b=c//2 and heads
[8g, 8g+8) with g=c%2.  Each core:
  - projects q,k,v for its batch against its 512 weight columns; RoPE is
    applied as rope(u) = u*cos + (R u)*sin where R (the +-1 pair-swap) is a
    constant 128x128 matmul on the projected tile,
  - computes S^T = K_rope Q_rope^T per head in [k, q] layout (column softmax:
    scores are O(5) so exp needs no max subtraction),
  - y^T = V^T P^T with a ones-column giving the softmax denominator for free,
  - reciprocal of the denominator via exp(-ln(x)) on ScalarE, broadcast to
    all partitions with a K=1 ones matmul,
  - writes the normalized attention weights w^T in [k, q] layout (contiguous
    HBM rows; the [q, k] layout would be a 4-byte scatter),
  - computes a partial output projection z = y @ Wo[rows of this core].
Host side: logits = z_even + z_odd per batch; w = w_t.swapaxes(-1, -2).

All matmuls run as float32r (1-8-11, full PE rate at free dim >= 256);
inputs are pre-rounded on the host, so accuracy vs the fp32 reference is
~5e-4 (the fp32r representation error).
"""

import numpy as np

import concourse.bass as bass
import concourse.tile as tile
import concourse.mybir as mybir
from concourse import bass_utils
from contextlib import ExitStack

B, T, D, H, HD = 4, 1024, 1024, 16, 64
THETA = 10000.0
P = 128
NCORES = 8
HPC = 8            # heads per core
DC = HPC * HD      # 512 projected dims per core
IT = D // P        # 8 contraction tiles
DT = DC // P       # 4 projected-dim tiles
QC = 2             # q chunks of 512
KT = T // P        # 8 k tiles
F32 = mybir.dt.float32
F32R = mybir.dt.float32r
AF = mybir.ActivationFunctionType


# --------------------------------------------------------------------------
# This neuronxcc/walrus build encodes at most ONE semaphore wait per TPB
# instruction and does not auto-split ("Too many sync wait commands").
# Post-process the scheduled BIR: hoist all-but-one wait of each instruction
# onto injected same-engine NoOps directly before it.
def _split_multi_waits(nc):
    n_new = 0
    for f in nc.m.functions:
        for blk in f.blocks:
            out = []
            changed = False
            for inst in blk.instructions:
                si = inst.sync_info
                waits = list(si.on_wait) if (si is not None and si.on_wait) else []
                if len(waits) > 1:
                    changed = True
                    for w in waits[:-1]:
                        nop = mybir.InstNoOp(
                            name=f"I-waitsplit-{n_new}",
                            engine=inst.engine,
                            ins=[],
                            outs=[],
                            sync_info=mybir.SyncInfo(on_wait=[w], on_update=[]),
                        )
                        n_new += 1
                        out.append(nop)
                    si.on_wait = [waits[-1]]
                out.append(inst)
            if changed:
                blk.instructions = out
    return n_new


class _TileContextFixed(tile.TileContext):
    """Plain TileContext; the wait split runs post-build (it confuses CoreSim,
    so simulation uses an unsplit build)."""


# --------------------------------------------------------------------------
def _emit_kernel(nc, tc, ctx, io):
    wpool = ctx.enter_context(tc.tile_pool(name="wpool", bufs=2))
    const = ctx.enter_context(tc.tile_pool(name="const", bufs=1))
    big = ctx.enter_context(tc.tile_pool(name="big", bufs=3))
    xp = ctx.enter_context(tc.tile_pool(name="xp", bufs=2))
    qkp = ctx.enter_context(tc.tile_pool(name="qkp", bufs=1))
    vhp = ctx.enter_context(tc.tile_pool(name="vhp", bufs=1))
    ytp = ctx.enter_context(tc.tile_pool(name="ytp", bufs=1))
    msc = ctx.enter_context(tc.tile_pool(name="msc", bufs=2))
    pp = ctx.enter_context(tc.tile_pool(name="pp", bufs=2, space="PSUM"))
    pyy = ctx.enter_context(tc.tile_pool(name="pyy", bufs=1, space="PSUM"))
    psm = ctx.enter_context(tc.tile_pool(name="psm", bufs=2, space="PSUM"))

    cos_sb = const.tile([P, T], F32)
    nc.scalar.dma_start(cos_sb[:], io["cosT"])
    sin_sb = const.tile([P, T], F32)
    nc.scalar.dma_start(sin_sb[:], io["sinT"])
    rmat_sb = const.tile([P, P], F32R)
    nc.scalar.dma_start(rmat_sb[:], io["rmat"])

    ones_sb = const.tile([P, P], F32R)
    # Memset can't write f32r in this toolchain; synthesize ones on ACT as
    # Identity(finite_input * 0 + 1).  cos_sb is loaded and finite.
    nc.scalar.activation(ones_sb[:], cos_sb[:, 0:P], AF.Identity,
                         bias=1.0, scale=0.0)

    # v with ones columns: per head pair the 130-wide slot holds
    # [vh_even(0:64) | one(64) | vh_odd(65:129) | one(129)].
    # even lhsT = cols [0:65), odd lhsT = cols [65:130); either way the
    # psum rows are y at 0-63 and the softmax denominator at row 64.
    vh_sb = vhp.tile([P, KT, DT, 130], F32R)
    vh2 = vh_sb[:].rearrange("p k d (two c) -> p k d two c", two=2)
    cos64 = cos_sb[:, 0:64].rearrange("p (k d two) -> p k d two", k=KT, d=DT)
    nc.scalar.activation(vh2[:, :, :, :, 64], cos64, AF.Identity,
                         bias=1.0, scale=0.0)

    qropeT = [qkp.tile([P, T], F32R, name=f"qropeT{i}") for i in range(DT)]
    kropeT = [qkp.tile([P, T], F32R, name=f"kropeT{i}") for i in range(DT)]
    yT = ytp.tile([P, DT, T], F32R)

    # ---- q/k projections with RoPE ---------------------------------------
    for xap, wap, outs in ((io["xqT"], io["wq"], qropeT),
                           (io["xkT"], io["wk"], kropeT)):
        wsb = wpool.tile([P, IT, DC], F32R, tag="w", name="wsb")
        nc.scalar.dma_start(wsb[:], wap.rearrange("(i p) d -> p i d", p=P))
        for qc in range(QC):
            qslice = slice(qc * 512, (qc + 1) * 512)
            xt = xp.tile([P, IT, 512], F32R, tag="xt", name="xt")
            nc.sync.dma_start(
                xt[:], xap.rearrange("(i p) t -> p i t", p=P)[:, :, qslice])
            for dt in range(DT):
                ps = pp.tile([P, 2, 512], F32, tag="pp", name="ps")
                for it in range(IT):
                    nc.tensor.matmul(
                        ps[:, 0, :],
                        wsb[:, it, dt * 128:(dt + 1) * 128],
                        xt[:, it, :],
                        start=(it == 0), stop=(it == IT - 1))
                # uh = plain projection (SBUF copy for the R matmul's rhs)
                uh = msc.tile([P, 512], F32R, tag="uh", name="uh")
                nc.scalar.copy(uh[:], ps[:, 0, :])
                # rotated projection: (R u) via constant matmul
                nc.tensor.matmul(ps[:, 1, :], rmat_sb[:], uh[:],
                                 start=True, stop=True)
                t1 = msc.tile([P, 512], F32, tag="t1", name="t1")
                t2 = msc.tile([P, 512], F32, tag="t2", name="t2")
                nc.vector.tensor_mul(t1[:], uh[:], cos_sb[:, qslice])
                nc.vector.tensor_mul(t2[:], ps[:, 1, :], sin_sb[:, qslice])
                nc.vector.tensor_add(outs[dt][:, qslice], t1[:], t2[:])

    # ---- v projection ----------------------------------------------------
    wv_sb = wpool.tile([P, IT, DC], F32R, tag="w", name="wv_sb")
    nc.scalar.dma_start(wv_sb[:], io["wv"].rearrange("(i p) d -> p i d", p=P))
    for half in range(2):
        xt = xp.tile([P, IT, 512], F32R, tag="xt", name="xt")
        nc.sync.dma_start(
            xt[:], io["xvT"].rearrange("(i p) t -> p i t", p=P)[:, :, half * 512:(half + 1) * 512])
        for k4 in range(4):
            kt = half * 4 + k4
            psv = psm.tile([P, 512], F32, tag="psm", name="psv")
            for it in range(IT):
                nc.tensor.matmul(
                    psv[:],
                    xt[:, it, k4 * 128:(k4 + 1) * 128],
                    wv_sb[:, it, :],
                    start=(it == 0), stop=(it == IT - 1))
            # d index = h*64+j, h = 2*g+e  ->  (g e j) with g=4 pairs, e=2, j=64
            pv = psv[:].rearrange("p (g e j) -> p g e j", g=DT, e=2)
            nc.scalar.copy(vh_sb[:, kt, :, 0:64], pv[:, :, 0, :])
            nc.scalar.copy(vh_sb[:, kt, :, 65:129], pv[:, :, 1, :])

    wo_sb = wpool.tile([P, DT, D], F32R, tag="w", name="wo_sb")
    nc.scalar.dma_start(wo_sb[:], io["wo"].rearrange("(t p) o -> p t o", p=P))

    # ---- attention per head ---------------------------------------------
    for h in range(HPC):
        dt = h // 2
        even = (h % 2 == 0)
        lane0 = 64 * (h % 2)
        lcol = 0 if even else 65
        psy = pyy.tile([P, QC, 512], F32, tag="psy", name="psy")
        pts = []
        for qc in range(QC):
            qslice = slice(qc * 512, (qc + 1) * 512)
            pt = big.tile([P, KT, 512], F32R, tag="big", name="pt")
            pts.append(pt)
            for k2 in range(4):
                pss = pp.tile([P, 2, 512], F32, tag="pp", name="pss")
                for j in range(2):
                    kt = k2 * 2 + j
                    nc.tensor.matmul(
                        pss[:, j, :],
                        kropeT[dt][lane0:lane0 + 64, kt * 128:(kt + 1) * 128],
                        qropeT[dt][lane0:lane0 + 64, qslice],
                        start=True, stop=True)
                nc.scalar.activation(pt[:, k2 * 2:k2 * 2 + 2, :], pss[:], AF.Exp,
                                     scale=0.125)
            for kt in range(KT):
                nc.tensor.matmul(
                    psy[0:65, qc, :],
                    vh_sb[:, kt, dt, lcol:lcol + 65],
                    pt[:, kt, :],
                    start=(kt == 0), stop=(kt == KT - 1))
        # reciprocal of the denominator rows: 1/x = exp(-ln(x)); the two
        # Ln ops are adjacent so ACT pays only 2 table switches per head
        lnrows, rrows = [], []
        for qc in range(QC):
            lnrow = msc.tile([P, 512], F32, tag="t2", name="lnrow")
            lnrows.append(lnrow)
            nc.scalar.activation(lnrow[64:65, :], psy[64:65, qc, :], AF.Ln)
        for qc in range(QC):
            rrow = msc.tile([P, 512], F32R, tag="uh", name="rrow")
            rrows.append(rrow)
            nc.scalar.activation(rrow[64:65, :], lnrows[qc][64:65, :], AF.Exp,
                                 scale=-1.0)
        pbrs = []
        for qc in range(QC):
            qslice = slice(qc * 512, (qc + 1) * 512)
            # broadcast the reciprocal row to all partitions: K=1 ones matmul
            pbr = psm.tile([P, 512], F32, tag="psm", name="pbr")
            pbrs.append(pbr)
            nc.tensor.matmul(pbr[:], ones_sb[64:65, :], rrows[qc][64:65, :],
                             start=True, stop=True)
            # normalized y into yT (head h occupies lanes lane0..lane0+64);
            # this (plus Ln) releases psy for the next head
            ystage = msc.tile([P, 512], F32, tag="t1", name="ystage")
            nc.vector.tensor_copy(ystage[0:64, :], psy[0:64, qc, :])
            if even:
                nc.vector.tensor_mul(yT[0:64, dt, qslice], ystage[0:64, :], pbr[0:64, :])
            else:
                stage = msc.tile([P, 512], F32R, tag="uh", name="stage")
                nc.vector.tensor_mul(stage[0:64, :], ystage[0:64, :], pbr[0:64, :])
                # partition relocate 0..64 -> 64..128 via SBUF-to-SBUF DMA
                nc.sync.dma_start(yT[64:128, dt, qslice], stage[0:64, :])
        for qc in range(QC):
            qslice = slice(qc * 512, (qc + 1) * 512)
            pt = pts[qc]
            # normalize w in place (one op) and store w^T (one 2MB DMA)
            rb = pbrs[qc][:].unsqueeze(1).to_broadcast([P, KT, 512])
            nc.vector.tensor_mul(pt[:], pt[:], rb)
            wdst = io["w_t"][h].rearrange("(kt p) q -> p kt q", p=P)
            nc.scalar.dma_start(wdst[:, :, qslice], pt[:])

    # ---- output projection (partial logits) ------------------------------
    for qt in range(8):
        zt = xp.tile([P, D], F32, tag="xt", name="zt")
        for oc in range(2):
            psz = psm.tile([P, 512], F32, tag="psm", name="psz")
            for dti in range(DT):
                nc.tensor.matmul(
                    psz[:],
                    yT[:, dti, qt * 128:(qt + 1) * 128],
                    wo_sb[:, dti, oc * 512:(oc + 1) * 512],
                    start=(dti == 0), stop=(dti == DT - 1))
            nc.vector.tensor_copy(zt[:, oc * 512:(oc + 1) * 512], psz[:])
        nc.scalar.dma_start(io["z"].rearrange("(qt p) o -> p qt o", p=P)[:, qt, :], zt[:])


def build_nc(split=True):
    nc = bass.Bass("TRN2", target_bir_lowering=False, debug=False)
    io = {
        "xqT": nc.dram_tensor("xqT", [D, T], F32R, kind="ExternalInput").ap(),
        "xkT": nc.dram_tensor("xkT", [D, T], F32R, kind="ExternalInput").ap(),
        "xvT": nc.dram_tensor("xvT", [D, T], F32R, kind="ExternalInput").ap(),
        "wq": nc.dram_tensor("wq", [D, DC], F32R, kind="ExternalInput").ap(),
        "wk": nc.dram_tensor("wk", [D, DC], F32R, kind="ExternalInput").ap(),
        "wv": nc.dram_tensor("wv", [D, DC], F32R, kind="ExternalInput").ap(),
        "wo": nc.dram_tensor("wo", [DC, D], F32R, kind="ExternalInput").ap(),
        "cosT": nc.dram_tensor("cosT", [P, T], F32, kind="ExternalInput").ap(),
        "sinT": nc.dram_tensor("sinT", [P, T], F32, kind="ExternalInput").ap(),
        "rmat": nc.dram_tensor("rmat", [P, P], F32R, kind="ExternalInput").ap(),
        "w_t": nc.dram_tensor("w_t", [HPC, T, T], F32R, kind="ExternalOutput").ap(),
        "z": nc.dram_tensor("z", [T, D], F32, kind="ExternalOutput").ap(),
    }
    with _TileContextFixed(nc) as tc, ExitStack() as ctx:
        _emit_kernel(nc, tc, ctx, io)
    if split:
        _split_multi_waits(nc)
    return nc


# --------------------------------------------------------------------------
def _rope_tables():
    inv_freq = 1.0 / (THETA ** (np.arange(0, HD, 2, dtype=np.float64) / HD))  # [32]
    pe = np.arange(T, dtype=np.float64)[:, None] * inv_freq[None, :]          # [T, 32]
    pe = np.repeat(pe, 2, axis=-1)                                            # [T, 64]
    cos = np.cos(pe).T.astype(np.float32)                                     # [64, T]
    sin = np.sin(pe).T.astype(np.float32)
    return (np.concatenate([cos, cos], axis=0).copy(),
            np.concatenate([sin, sin], axis=0).copy())


def _rot_matrix():
    # psB = R @ u with R[2i, 2i+1] = -1, R[2i+1, 2i] = +1;
    # lhsT layout: rmat[d, d'] = R[d', d]
    rm = np.zeros((P, P), np.float32)
    ev = np.arange(0, P, 2)
    rm[ev + 1, ev] = -1.0
    rm[ev, ev + 1] = 1.0
    return rm


def _round_f32r(a):
    """Round fp32 to fp32r (1-8-11: low 12 mantissa bits zero), RTN-even."""
    bits = np.ascontiguousarray(a, np.float32).view(np.uint32)
    r = (bits + np.uint32(0x7FF) + ((bits >> np.uint32(12)) & np.uint32(1))) & np.uint32(0xFFFFF000)
    return r.view(np.float32)


def make_in_maps(q, k, v, Wq, Wk, Wv, Wo):
    cosT, sinT = _rope_tables()
    rmat = _rot_matrix()
    in_maps = []
    for c in range(NCORES):
        b, g = c // 2, c % 2
        cols = slice(g * DC, (g + 1) * DC)
        in_maps.append({
            "xqT": _round_f32r(q[b].T),
            "xkT": _round_f32r(k[b].T),
            "xvT": _round_f32r(v[b].T),
            "wq": _round_f32r(Wq[:, cols]),
            "wk": _round_f32r(Wk[:, cols]),
            "wv": _round_f32r(Wv[:, cols]),
            "wo": _round_f32r(Wo[cols, :]),
            "cosT": cosT,
            "sinT": sinT,
            "rmat": rmat,
        })
    return in_maps


def assemble(results):
    logits = np.empty((B, T, D), np.float32)
    w = np.empty((B, H, T, T), np.float32)
    for b in range(B):
        logits[b] = results[2 * b]["z"] + results[2 * b + 1]["z"]
        for g in range(2):
            w_t = results[2 * b + g]["w_t"]          # [HPC, k, q]
            w[b, 8 * g:8 * g + 8] = w_t.swapaxes(-1, -2)
    return logits, w


_NC = None


def kernel(q, k, v, Wq, Wk, Wv, Wo):
    global _NC
    q, k, v = (np.asarray(x, np.float32) for x in (q, k, v))
    Wq, Wk, Wv, Wo = (np.asarray(x, np.float32) for x in (Wq, Wk, Wv, Wo))
    if _NC is None:
        _NC = build_nc()
    in_maps = make_in_maps(q, k, v, Wq, Wk, Wv, Wo)
    res = bass_utils.run_bass_kernel_spmd(_NC, in_maps, core_ids=list(range(NCORES)))
    return assemble(res.results)


# revision 24
# speedup vs baseline: 1.1161x; 1.1161x over previous
"""Multi-head attention with RoPE (B=4, T=1024, D=1024, H=16) on 8 TRN2
NeuronCores.

Sharding: 64 (batch, head) units -> core c handles batch b=c//2 and heads
[8g, 8g+8) with g=c%2.  Each core:
  - projects q,k,v for its batch against its 512 weight columns; RoPE is
    applied as rope(u) = u*cos + (R u)*sin where R (the +-1 pair-swap) is a
    constant 128x128 matmul on the projected tile,
  - computes S^T = K_rope Q_rope^T per head in [k, q] layout (column softmax:
    scores are O(5) so exp needs no max subtraction),
  - y^T = V^T P^T with a ones-column giving the softmax denominator for free,
  - reciprocal of the denominator via exp(-ln(x)) on ScalarE, broadcast to
    all partitions with a K=1 ones matmul,
  - writes the normalized attention weights w^T in [k, q] layout (contiguous
    HBM rows; the [q, k] layout would be a 4-byte scatter),
  - computes a partial output projection z = y @ Wo[rows of this core].
Host side: logits = z_even + z_odd per batch; w = w_t.swapaxes(-1, -2).

All matmuls run as float32r (1-8-11, full PE rate at free dim >= 256);
inputs are pre-rounded on the host, so accuracy vs the fp32 reference is
~5e-4 (the fp32r representation error).
"""

import numpy as np

import concourse.bass as bass
import concourse.tile as tile
import concourse.mybir as mybir
from concourse import bass_utils
from contextlib import ExitStack

B, T, D, H, HD = 4, 1024, 1024, 16, 64
THETA = 10000.0
P = 128
NCORES = 8
HPC = 8            # heads per core
DC = HPC * HD      # 512 projected dims per core
IT = D // P        # 8 contraction tiles
DT = DC // P       # 4 projected-dim tiles
QC = 2             # q chunks of 512
KT = T // P        # 8 k tiles
F32 = mybir.dt.float32
F32R = mybir.dt.float32r
AF = mybir.ActivationFunctionType


# --------------------------------------------------------------------------
# This neuronxcc/walrus build encodes at most ONE semaphore wait per TPB
# instruction and does not auto-split ("Too many sync wait commands").
# Post-process the scheduled BIR: hoist all-but-one wait of each instruction
# onto injected same-engine NoOps directly before it.
def _split_multi_waits(nc):
    n_new = 0
    for f in nc.m.functions:
        for blk in f.blocks:
            out = []
            changed = False
            for inst in blk.instructions:
                si = inst.sync_info
                waits = list(si.on_wait) if (si is not None and si.on_wait) else []
                if len(waits) > 1:
                    changed = True
                    for w in waits[:-1]:
                        nop = mybir.InstNoOp(
                            name=f"I-waitsplit-{n_new}",
                            engine=inst.engine,
                            ins=[],
                            outs=[],
                            sync_info=mybir.SyncInfo(on_wait=[w], on_update=[]),
                        )
                        n_new += 1
                        out.append(nop)
                    si.on_wait = [waits[-1]]
                out.append(inst)
            if changed:
                blk.instructions = out
    return n_new


class _TileContextFixed(tile.TileContext):
    """Plain TileContext; the wait split runs post-build (it confuses CoreSim,
    so simulation uses an unsplit build)."""


# --------------------------------------------------------------------------
def _emit_kernel(nc, tc, ctx, io):
    wpool = ctx.enter_context(tc.tile_pool(name="wpool", bufs=2))
    const = ctx.enter_context(tc.tile_pool(name="const", bufs=1))
    big = ctx.enter_context(tc.tile_pool(name="big", bufs=4))
    xp = ctx.enter_context(tc.tile_pool(name="xp", bufs=3))
    qkp = ctx.enter_context(tc.tile_pool(name="qkp", bufs=1))
    vhp = ctx.enter_context(tc.tile_pool(name="vhp", bufs=1))
    ytp = ctx.enter_context(tc.tile_pool(name="ytp", bufs=1))
    msc = ctx.enter_context(tc.tile_pool(name="msc", bufs=2))
    pp = ctx.enter_context(tc.tile_pool(name="pp", bufs=2, space="PSUM"))
    pyy = ctx.enter_context(tc.tile_pool(name="pyy", bufs=1, space="PSUM"))
    psm = ctx.enter_context(tc.tile_pool(name="psm", bufs=2, space="PSUM"))

    cos_sb = const.tile([P, T], F32)
    nc.scalar.dma_start(cos_sb[:], io["cosT"])
    sin_sb = const.tile([P, T], F32)
    nc.scalar.dma_start(sin_sb[:], io["sinT"])
    rmat_sb = const.tile([P, P], F32R)
    nc.scalar.dma_start(rmat_sb[:], io["rmat"])

    ones_sb = const.tile([P, P], F32R)
    # Memset can't write f32r in this toolchain; synthesize ones on ACT as
    # Identity(finite_input * 0 + 1).  cos_sb is loaded and finite.
    nc.scalar.activation(ones_sb[:], cos_sb[:, 0:P], AF.Identity,
                         bias=1.0, scale=0.0)

    # v with ones columns: per head pair the 130-wide slot holds
    # [vh_even(0:64) | one(64) | vh_odd(65:129) | one(129)].
    # even lhsT = cols [0:65), odd lhsT = cols [65:130); either way the
    # psum rows are y at 0-63 and the softmax denominator at row 64.
    vh_sb = vhp.tile([P, KT, DT, 130], F32R)
    vh2 = vh_sb[:].rearrange("p k d (two c) -> p k d two c", two=2)
    cos64 = cos_sb[:, 0:64].rearrange("p (k d two) -> p k d two", k=KT, d=DT)
    nc.scalar.activation(vh2[:, :, :, :, 64], cos64, AF.Identity,
                         bias=1.0, scale=0.0)

    qropeT = [qkp.tile([P, T], F32R, name=f"qropeT{i}") for i in range(DT)]
    kropeT = [qkp.tile([P, T], F32R, name=f"kropeT{i}") for i in range(DT)]
    yT = ytp.tile([P, DT, T], F32R)

    # ---- q/k projections with RoPE ---------------------------------------
    for xap, wap, outs in ((io["xqT"], io["wq"], qropeT),
                           (io["xkT"], io["wk"], kropeT)):
        wsb = wpool.tile([P, IT, DC], F32R, tag="w", name="wsb")
        nc.scalar.dma_start(wsb[:], wap.rearrange("(i p) d -> p i d", p=P))
        for qc in range(QC):
            qslice = slice(qc * 512, (qc + 1) * 512)
            xts = []
            for ih in range(2):
                xth = xp.tile([P, IT // 2, 512], F32R, tag="xt", name="xt")
                xts.append(xth)
                nc.sync.dma_start(
                    xth[:], xap.rearrange("(i p) t -> p i t", p=P)[:, ih * 4:(ih + 1) * 4, qslice])
            for dt in range(DT):
                ps = pp.tile([P, 2, 512], F32, tag="pp", name="ps")
                for it in range(IT):
                    nc.tensor.matmul(
                        ps[:, 0, :],
                        wsb[:, it, dt * 128:(dt + 1) * 128],
                        xts[it // 4][:, it % 4, :],
                        start=(it == 0), stop=(it == IT - 1))
                # uh = plain projection (SBUF copy for the R matmul's rhs)
                uh = msc.tile([P, 512], F32R, tag="uh", name="uh")
                nc.scalar.copy(uh[:], ps[:, 0, :])
                # rotated projection: (R u) via constant matmul
                nc.tensor.matmul(ps[:, 1, :], rmat_sb[:], uh[:],
                                 start=True, stop=True)
                t1 = msc.tile([P, 512], F32, tag="t1", name="t1")
                t2 = msc.tile([P, 512], F32, tag="t2", name="t2")
                nc.vector.tensor_mul(t1[:], uh[:], cos_sb[:, qslice])
                nc.vector.tensor_mul(t2[:], ps[:, 1, :], sin_sb[:, qslice])
                nc.vector.tensor_add(outs[dt][:, qslice], t1[:], t2[:])

    # ---- v projection ----------------------------------------------------
    wv_sb = wpool.tile([P, IT, DC], F32R, tag="w", name="wv_sb")
    nc.scalar.dma_start(wv_sb[:], io["wv"].rearrange("(i p) d -> p i d", p=P))
    for half in range(2):
        xts = []
        for ih in range(2):
            xth = xp.tile([P, IT // 2, 512], F32R, tag="xt", name="xt")
            xts.append(xth)
            nc.sync.dma_start(
                xth[:], io["xvT"].rearrange("(i p) t -> p i t", p=P)[:, ih * 4:(ih + 1) * 4, half * 512:(half + 1) * 512])
        for k4 in range(4):
            kt = half * 4 + k4
            psv = psm.tile([P, 512], F32, tag="psm", name="psv")
            for it in range(IT):
                nc.tensor.matmul(
                    psv[:],
                    xts[it // 4][:, it % 4, k4 * 128:(k4 + 1) * 128],
                    wv_sb[:, it, :],
                    start=(it == 0), stop=(it == IT - 1))
            # d index = h*64+j, h = 2*g+e  ->  (g e j) with g=4 pairs, e=2, j=64
            pv = psv[:].rearrange("p (g e j) -> p g e j", g=DT, e=2)
            nc.scalar.copy(vh_sb[:, kt, :, 0:64], pv[:, :, 0, :])
            nc.scalar.copy(vh_sb[:, kt, :, 65:129], pv[:, :, 1, :])

    wo_sb = wpool.tile([P, DT, D], F32R, tag="w", name="wo_sb")
    nc.scalar.dma_start(wo_sb[:], io["wo"].rearrange("(t p) o -> p t o", p=P))

    # ---- attention per head ---------------------------------------------
    for h in range(HPC):
        dt = h // 2
        even = (h % 2 == 0)
        lane0 = 64 * (h % 2)
        lcol = 0 if even else 65
        psy = pyy.tile([P, QC, 512], F32, tag="psy", name="psy")
        pts = []
        for qc in range(QC):
            qslice = slice(qc * 512, (qc + 1) * 512)
            pt = big.tile([P, KT, 512], F32R, tag="big", name="pt")
            pts.append(pt)
            for k2 in range(4):
                pss = pp.tile([P, 2, 512], F32, tag="pp", name="pss")
                for j in range(2):
                    kt = k2 * 2 + j
                    nc.tensor.matmul(
                        pss[:, j, :],
                        kropeT[dt][lane0:lane0 + 64, kt * 128:(kt + 1) * 128],
                        qropeT[dt][lane0:lane0 + 64, qslice],
                        start=True, stop=True)
                nc.scalar.activation(pt[:, k2 * 2:k2 * 2 + 2, :], pss[:], AF.Exp,
                                     scale=0.125)
            for kt in range(KT):
                nc.tensor.matmul(
                    psy[0:65, qc, :],
                    vh_sb[:, kt, dt, lcol:lcol + 65],
                    pt[:, kt, :],
                    start=(kt == 0), stop=(kt == KT - 1))
        # reciprocal of the denominator rows: 1/x = exp(-ln(x)); the two
        # Ln ops are adjacent so ACT pays only 2 table switches per head
        lnrows, rrows = [], []
        for qc in range(QC):
            lnrow = msc.tile([P, 512], F32, tag="t2", name="lnrow")
            lnrows.append(lnrow)
            nc.scalar.activation(lnrow[64:65, :], psy[64:65, qc, :], AF.Ln)
        for qc in range(QC):
            rrow = msc.tile([P, 512], F32R, tag="uh", name="rrow")
            rrows.append(rrow)
            nc.scalar.activation(rrow[64:65, :], lnrows[qc][64:65, :], AF.Exp,
                                 scale=-1.0)
        pbrs = []
        for qc in range(QC):
            qslice = slice(qc * 512, (qc + 1) * 512)
            # broadcast the reciprocal row to all partitions: K=1 ones matmul
            pbr = psm.tile([P, 512], F32, tag="psm", name="pbr")
            pbrs.append(pbr)
            nc.tensor.matmul(pbr[:], ones_sb[64:65, :], rrows[qc][64:65, :],
                             start=True, stop=True)
            # normalized y into yT (head h occupies lanes lane0..lane0+64);
            # this (plus Ln) releases psy for the next head
            ystage = msc.tile([P, 512], F32, tag="t1", name="ystage")
            nc.scalar.copy(ystage[0:64, :], psy[0:64, qc, :])
            if even:
                nc.vector.tensor_mul(yT[0:64, dt, qslice], ystage[0:64, :], pbr[0:64, :])
            else:
                stage = msc.tile([P, 512], F32R, tag="uh", name="stage")
                nc.vector.tensor_mul(stage[0:64, :], ystage[0:64, :], pbr[0:64, :])
                # partition relocate 0..64 -> 64..128 via SBUF-to-SBUF DMA
                nc.sync.dma_start(yT[64:128, dt, qslice], stage[0:64, :])
        for qc in range(QC):
            qslice = slice(qc * 512, (qc + 1) * 512)
            pt = pts[qc]
            # normalize w in place (one op) and store w^T (one 2MB DMA)
            rb = pbrs[qc][:].unsqueeze(1).to_broadcast([P, KT, 512])
            nc.vector.tensor_mul(pt[:], pt[:], rb)
            wdst = io["w_t"][h].rearrange("(kt p) q -> p kt q", p=P)
            nc.scalar.dma_start(wdst[:, :, qslice], pt[:])

    # ---- output projection (partial logits) ------------------------------
    for qt in range(8):
        zt = big.tile([P, D], F32, tag="big", name="zt")
        for oc in range(2):
            psz = psm.tile([P, 512], F32, tag="psm", name="psz")
            for dti in range(DT):
                nc.tensor.matmul(
                    psz[:],
                    yT[:, dti, qt * 128:(qt + 1) * 128],
                    wo_sb[:, dti, oc * 512:(oc + 1) * 512],
                    start=(dti == 0), stop=(dti == DT - 1))
            nc.vector.tensor_copy(zt[:, oc * 512:(oc + 1) * 512], psz[:])
        nc.scalar.dma_start(io["z"].rearrange("(qt p) o -> p qt o", p=P)[:, qt, :], zt[:])


def build_nc(split=True):
    nc = bass.Bass("TRN2", target_bir_lowering=False, debug=False)
    io = {
        "xqT": nc.dram_tensor("xqT", [D, T], F32R, kind="ExternalInput").ap(),
        "xkT": nc.dram_tensor("xkT", [D, T], F32R, kind="ExternalInput").ap(),
        "xvT": nc.dram_tensor("xvT", [D, T], F32R, kind="ExternalInput").ap(),
        "wq": nc.dram_tensor("wq", [D, DC], F32R, kind="ExternalInput").ap(),
        "wk": nc.dram_tensor("wk", [D, DC], F32R, kind="ExternalInput").ap(),
        "wv": nc.dram_tensor("wv", [D, DC], F32R, kind="ExternalInput").ap(),
        "wo": nc.dram_tensor("wo", [DC, D], F32R, kind="ExternalInput").ap(),
        "cosT": nc.dram_tensor("cosT", [P, T], F32, kind="ExternalInput").ap(),
        "sinT": nc.dram_tensor("sinT", [P, T], F32, kind="ExternalInput").ap(),
        "rmat": nc.dram_tensor("rmat", [P, P], F32R, kind="ExternalInput").ap(),
        "w_t": nc.dram_tensor("w_t", [HPC, T, T], F32R, kind="ExternalOutput").ap(),
        "z": nc.dram_tensor("z", [T, D], F32, kind="ExternalOutput").ap(),
    }
    with _TileContextFixed(nc) as tc, ExitStack() as ctx:
        _emit_kernel(nc, tc, ctx, io)
    if split:
        _split_multi_waits(nc)
    return nc


# --------------------------------------------------------------------------
def _rope_tables():
    inv_freq = 1.0 / (THETA ** (np.arange(0, HD, 2, dtype=np.float64) / HD))  # [32]
    pe = np.arange(T, dtype=np.float64)[:, None] * inv_freq[None, :]          # [T, 32]
    pe = np.repeat(pe, 2, axis=-1)                                            # [T, 64]
    cos = np.cos(pe).T.astype(np.float32)                                     # [64, T]
    sin = np.sin(pe).T.astype(np.float32)
    return (np.concatenate([cos, cos], axis=0).copy(),
            np.concatenate([sin, sin], axis=0).copy())


def _rot_matrix():
    # psB = R @ u with R[2i, 2i+1] = -1, R[2i+1, 2i] = +1;
    # lhsT layout: rmat[d, d'] = R[d', d]
    rm = np.zeros((P, P), np.float32)
    ev = np.arange(0, P, 2)
    rm[ev + 1, ev] = -1.0
    rm[ev, ev + 1] = 1.0
    return rm


def _round_f32r(a):
    """Round fp32 to fp32r (1-8-11: low 12 mantissa bits zero), RTN-even."""
    bits = np.ascontiguousarray(a, np.float32).view(np.uint32)
    r = (bits + np.uint32(0x7FF) + ((bits >> np.uint32(12)) & np.uint32(1))) & np.uint32(0xFFFFF000)
    return r.view(np.float32)


def make_in_maps(q, k, v, Wq, Wk, Wv, Wo):
    cosT, sinT = _rope_tables()
    rmat = _rot_matrix()
    in_maps = []
    for c in range(NCORES):
        b, g = c // 2, c % 2
        cols = slice(g * DC, (g + 1) * DC)
        in_maps.append({
            "xqT": _round_f32r(q[b].T),
            "xkT": _round_f32r(k[b].T),
            "xvT": _round_f32r(v[b].T),
            "wq": _round_f32r(Wq[:, cols]),
            "wk": _round_f32r(Wk[:, cols]),
            "wv": _round_f32r(Wv[:, cols]),
            "wo": _round_f32r(Wo[cols, :]),
            "cosT": cosT,
            "sinT": sinT,
            "rmat": rmat,
        })
    return in_maps


def assemble(results):
    logits = np.empty((B, T, D), np.float32)
    w = np.empty((B, H, T, T), np.float32)
    for b in range(B):
        logits[b] = results[2 * b]["z"] + results[2 * b + 1]["z"]
        for g in range(2):
            w_t = results[2 * b + g]["w_t"]          # [HPC, k, q]
            w[b, 8 * g:8 * g + 8] = w_t.swapaxes(-1, -2)
    return logits, w


_NC = None


def kernel(q, k, v, Wq, Wk, Wv, Wo):
    global _NC
    q, k, v = (np.asarray(x, np.float32) for x in (q, k, v))
    Wq, Wk, Wv, Wo = (np.asarray(x, np.float32) for x in (Wq, Wk, Wv, Wo))
    if _NC is None:
        _NC = build_nc()
    in_maps = make_in_maps(q, k, v, Wq, Wk, Wv, Wo)
    res = bass_utils.run_bass_kernel_spmd(_NC, in_maps, core_ids=list(range(NCORES)))
    return assemble(res.results)
